# revision 2
# baseline (speedup 1.0000x reference)
"""Trainium2 Bass kernel for nn_CAMLocalHead (CAM target + conv head + BCE).

Self-contained: takes FULL inputs, shards batch B=8 across 8 NeuronCores
(one sample per core), runs a Bass/Tile kernel per core, sums the per-core
partial BCE sums on host.
"""
import sys

for _p in ("/opt/trn_rl_repo", "/opt/pypackages"):
    if _p not in sys.path:
        sys.path.append(_p)

import numpy as np
import ml_dtypes

# Problem dims (hardcoded per spec)
B, C, T, H, W = 8, 2048, 16, 7, 7
K, D = 400, 512
N_TOKEN = 392
P = 128
CT = C // P          # 16 c-tiles
DT = D // P          # 4 d-tiles
NH = 2               # spatial halves (t 0..7, 8..15)
TH = T // NH         # 8
PLANE = 81           # 9x9 padded plane
NF = TH * H * W      # 392 positions per half
NPOS = T * H * W     # 784
PADN = 7 * P         # 896 (784 padded to 7 chunks of 128)
NEG = -1.0e30

_cache = {}


def _build_nc():
    import concourse.bacc as bacc
    import concourse.mybir as mybir
    from concourse import tile

    f32 = mybir.dt.float32
    bf16 = mybir.dt.bfloat16
    AX = mybir.AxisListType.X
    OP = mybir.AluOpType
    AF = mybir.ActivationFunctionType

    nc = bacc.Bacc(trn_type="TRN2")

    xpad_d = nc.dram_tensor("xpad", [CT, P, T * PLANE], bf16, kind="ExternalInput")
    wt_d = nc.dram_tensor("wt", [DT, 9, P, CT * P], bf16, kind="ExternalInput")
    proj_d = nc.dram_tensor("proj", [K, C], bf16, kind="ExternalInput")
    xfp_d = nc.dram_tensor("xfp", [1, K], f32, kind="ExternalInput")
    cb_d = nc.dram_tensor("cb", [P, DT], f32, kind="ExternalInput")
    sw_d = nc.dram_tensor("sw", [P, DT], bf16, kind="ExternalInput")
    sb_d = nc.dram_tensor("sb", [1, 1], f32, kind="ExternalInput")
    out_d = nc.dram_tensor("out", [1, 1], f32, kind="ExternalOutput")

    with tile.TileContext(nc) as tc:
        with (
            tc.tile_pool(name="const", bufs=1) as cp,
            tc.tile_pool(name="wp", bufs=3) as wp,
            tc.tile_pool(name="rp", bufs=4) as rp,
            tc.tile_pool(name="cps", bufs=2, space="PSUM") as cps,
            tc.tile_pool(name="sps", bufs=1, space="PSUM") as sps,
            tc.tile_pool(name="mps", bufs=2, space="PSUM") as mps,
        ):
            # ---------- constants / inputs resident in SBUF ----------
            xall = cp.tile([P, CT * T * PLANE], bf16)
            for ct in range(CT):
                nc.sync.dma_start(
                    xall[:, ct * T * PLANE:(ct + 1) * T * PLANE], xpad_d[ct])

            proj_sb = cp.tile([P, 4 * C], bf16)
            for kc in range(4):
                kcnt = min(P, K - kc * P)
                nc.sync.dma_start(
                    proj_sb[0:kcnt, kc * C:(kc + 1) * C],
                    proj_d[kc * P:kc * P + kcnt, :])

            xfp = cp.tile([1, K], f32)
            nc.sync.dma_start(xfp[:], xfp_d[:])
            cb_sb = cp.tile([P, DT], f32)
            nc.sync.dma_start(cb_sb[:], cb_d[:])
            sw_sb = cp.tile([P, DT], bf16)
            nc.sync.dma_start(sw_sb[:], sw_d[:])
            sb_sb = cp.tile([1, 1], f32)
            nc.sync.dma_start(sb_sb[:], sb_d[:])

            ones11 = cp.tile([1, 1], f32)
            nc.vector.memset(ones11[:], 1.0)
            ones_row = cp.tile([1, P], f32)
            nc.vector.memset(ones_row[:], 1.0)
            ones_col = cp.tile([P, 1], f32)
            nc.vector.memset(ones_col[:], 1.0)

            xv = xall[:].rearrange(
                "p (ct t h w) -> p ct t h w", ct=CT, t=T, h=9, w=9)

            def xview(ct, tap, nh):
                dh, dw = tap // 3, tap % 3
                return xv[:, ct, nh * TH:(nh + 1) * TH, dh:dh + 7, dw:dw + 7]

            # ---------- CAM front-end ----------
            # argmax class via one-hot (sigmoid is monotonic -> argmax on raw)
            m = cp.tile([1, 1], f32)
            nc.vector.reduce_max(m[:], xfp[:], axis=AX)
            oh = cp.tile([1, 4 * P], f32)
            nc.vector.memset(oh[:], 0.0)
            nc.vector.tensor_scalar(oh[0:1, 0:K], xfp[:], m[:], None,
                                    op0=OP.is_equal)
            ohT_ps = mps.tile([P, 4], f32, tag="mp")
            for i in range(4):
                nc.tensor.transpose(ohT_ps[:, i:i + 1],
                                    oh[0:1, i * P:(i + 1) * P], ones11[:])
            ohT = cp.tile([P, 4], bf16)
            nc.vector.tensor_copy(ohT[:], ohT_ps[:])

            # w_selT[c] = proj_weight[top_cls, c], as [128, CT] (c-tile cols)
            wps = mps.tile([P, CT], f32, tag="mp")
            for ct in range(CT):
                for kc in range(4):
                    kcnt = min(P, K - kc * P)
                    nc.tensor.matmul(
                        wps[:, ct:ct + 1],
                        proj_sb[0:kcnt, kc * C + ct * P:kc * C + (ct + 1) * P],
                        ohT[0:kcnt, kc:kc + 1],
                        start=(kc == 0), stop=(kc == 3))
            wsel = cp.tile([P, CT], bf16)
            nc.vector.tensor_copy(wsel[:], wps[:])

            # cam[1, 784] = w_sel @ x  (center view of padded x)
            cam_ps = [mps.tile([1, NF], f32, tag="mp", name=f"cam_ps{_nh}")
                      for _nh in range(NH)]
            for nh in range(NH):
                for ct in range(CT):
                    nc.tensor.matmul(
                        cam_ps[nh][:], wsel[:, ct:ct + 1], xview(ct, 4, nh),
                        start=(ct == 0), stop=(ct == CT - 1))
            cam_row = cp.tile([1, PADN], f32)
            for nh in range(NH):
                nc.vector.tensor_copy(
                    cam_row[0:1, nh * NF:(nh + 1) * NF], cam_ps[nh][:])

            cmin = cp.tile([1, 1], f32)
            cmax = cp.tile([1, 1], f32)
            nc.vector.tensor_reduce(cmin[:], cam_row[0:1, 0:NPOS], axis=AX,
                                    op=OP.min)
            nc.vector.reduce_max(cmax[:], cam_row[0:1, 0:NPOS], axis=AX)
            rng_t = cp.tile([1, 1], f32)
            nc.vector.tensor_scalar(rng_t[:], cmax[:], cmin[:], None,
                                    op0=OP.subtract)
            inv = cp.tile([1, 1], f32)
            nc.vector.reciprocal(inv[:], rng_t[:])

            camn = cp.tile([1, PADN], f32)
            nc.vector.memset(camn[:], NEG)
            nc.vector.tensor_scalar(camn[0:1, 0:NPOS], cam_row[0:1, 0:NPOS],
                                    cmin[:], inv[:],
                                    op0=OP.subtract, op1=OP.mult)

            # broadcast camn across partitions: camB[128, 784]
            camB = cp.tile([P, NPOS], f32)
            for nh in range(NH):
                cb_ps = mps.tile([P, NF], f32, tag="mp")
                nc.tensor.matmul(cb_ps[:], ones_row[:],
                                 camn[0:1, nh * NF:(nh + 1) * NF],
                                 start=True, stop=True)
                nc.vector.tensor_copy(camB[:, nh * NF:(nh + 1) * NF], cb_ps[:])

            # camn in partition layout [128, 7]
            cnp_ps = mps.tile([P, 7], f32, tag="mp")
            for a in range(7):
                nc.tensor.transpose(cnp_ps[:, a:a + 1],
                                    camn[0:1, a * P:(a + 1) * P], ones11[:])
            camnP = cp.tile([P, 7], f32)
            nc.vector.tensor_copy(camnP[:], cnp_ps[:])

            # rank[p,a] = #{j : camn[j] >= camn[p,a]}; top-392 mask = rank<=392
            ge = cp.tile([P, NPOS], f32)
            rank = cp.tile([P, 7], f32)
            for a in range(7):
                nc.vector.tensor_scalar(ge[:], camB[:], camnP[:, a:a + 1],
                                        None, op0=OP.is_ge, op1=OP.add,
                                        accum_out=rank[:, a:a + 1])
            maskP = cp.tile([P, 7], f32)
            nc.vector.tensor_scalar(maskP[:], rank[:], float(N_TOKEN), None,
                                    op0=OP.is_le)
            yP = cp.tile([P, 7], f32)
            nc.vector.tensor_mul(yP[:], maskP[:], camnP[:])

            # ---------- conv main loop ----------
            s_ps = [sps.tile([1, NF], f32, tag=f"s{nh}", name=f"s_ps{nh}")
                    for nh in range(NH)]
            for dt in range(DT):
                ps = [cps.tile([P, NF], f32, tag=f"cv{nh}", name=f"ps{dt}_{nh}")
                      for nh in range(NH)]
                for tap in range(9):
                    w_all = wp.tile([P, CT * P], bf16)
                    nc.sync.dma_start(w_all[:], wt_d[dt, tap])
                    for ct in range(CT):
                        lhsT = w_all[:, ct * P:(ct + 1) * P]
                        for nh in range(NH):
                            nc.tensor.matmul(
                                ps[nh][:], lhsT, xview(ct, tap, nh),
                                start=(tap == 0 and ct == 0),
                                stop=(tap == 8 and ct == CT - 1))
                for nh in range(NH):
                    relu_t = rp.tile([P, NF], bf16)
                    nc.scalar.activation(relu_t[:], ps[nh][:], AF.Relu,
                                         bias=cb_sb[:, dt:dt + 1])
                    nc.tensor.matmul(s_ps[nh][:], sw_sb[:, dt:dt + 1],
                                     relu_t[:],
                                     start=(dt == 0), stop=(dt == DT - 1))

            # ---------- epilogue: BCE = sum softplus(xcam) - sum xcam*y ----
            xcam_row = cp.tile([1, PADN], f32)
            nc.vector.memset(xcam_row[:], 0.0)
            for nh in range(NH):
                nc.vector.tensor_scalar(
                    xcam_row[0:1, nh * NF:(nh + 1) * NF], s_ps[nh][:],
                    sb_sb[:], None, op0=OP.add)

            et = cp.tile([1, NPOS], f32)
            nc.scalar.activation(et[:], xcam_row[0:1, 0:NPOS], AF.Exp)
            sp = cp.tile([1, NPOS], f32)
            sp_sum = cp.tile([1, 1], f32)
            nc.scalar.activation(sp[:], et[:], AF.Ln, bias=1.0,
                                 accum_out=sp_sum[:])

            xcp_ps = mps.tile([P, 7], f32, tag="mp")
            for a in range(7):
                nc.tensor.transpose(xcp_ps[:, a:a + 1],
                                    xcam_row[0:1, a * P:(a + 1) * P],
                                    ones11[:])
            xcamP = cp.tile([P, 7], f32)
            nc.vector.tensor_copy(xcamP[:], xcp_ps[:])

            prodP = cp.tile([P, 7], f32)
            nc.vector.tensor_mul(prodP[:], yP[:], xcamP[:])
            partial = cp.tile([P, 1], f32)
            nc.vector.reduce_sum(partial[:], prodP[:], axis=AX)

            dot_ps = mps.tile([1, 1], f32, tag="mp")
            nc.tensor.matmul(dot_ps[:], ones_col[:], partial[:],
                             start=True, stop=True)

            final = cp.tile([1, 1], f32)
            nc.vector.tensor_scalar(final[:], dot_ps[:], -1.0, sp_sum[:],
                                    op0=OP.mult, op1=OP.add)
            nc.sync.dma_start(out_d[:], final[:])

    nc.compile()
    return nc


def _prep_in_maps(x, x_fpv_pred, proj_weight, conv1_w, conv1_b, score_w,
                  score_b):
    bf16 = ml_dtypes.bfloat16
    xr = np.asarray(x, np.float32).reshape(B, CT, P, T, H, W)
    xpad = np.zeros((B, CT, P, T, 9, 9), dtype=bf16)
    xpad[:, :, :, :, 1:8, 1:8] = xr.astype(bf16)
    xpad = np.ascontiguousarray(xpad.reshape(B, CT, P, T * PLANE))

    w9 = np.asarray(conv1_w, np.float32).reshape(D, C, 9)
    # wt[dt, tap, p, ct*P + q] = conv1_w[dt*P+q, ct*P+p, tap]
    wt = np.ascontiguousarray(
        w9.reshape(DT, P, CT, P, 9).transpose(0, 4, 3, 2, 1)
        .reshape(DT, 9, P, CT * P)).astype(bf16)

    proj_bf = np.asarray(proj_weight, np.float32).astype(bf16)
    cb = np.ascontiguousarray(
        np.asarray(conv1_b, np.float32).reshape(DT, P).T)
    sw = np.ascontiguousarray(
        np.asarray(score_w, np.float32).reshape(DT, P).T).astype(bf16)
    sb = np.asarray(score_b, np.float32).reshape(1, 1)
    xfp = np.asarray(x_fpv_pred, np.float32)

    in_maps = []
    for b in range(B):
        in_maps.append({
            "xpad": xpad[b],
            "wt": wt,
            "proj": proj_bf,
            "xfp": np.ascontiguousarray(xfp[b:b + 1]),
            "cb": cb,
            "sw": sw,
            "sb": sb,
        })
    return in_maps


def run(inputs, trace=False):
    """Build (cached), run on 8 cores, return (loss, BassKernelResults)."""
    from concourse.bass_utils import run_bass_kernel_spmd

    if "nc" not in _cache:
        _cache["nc"] = _build_nc()
    nc = _cache["nc"]
    in_maps = _prep_in_maps(**inputs)
    res = run_bass_kernel_spmd(nc, in_maps, core_ids=list(range(B)),
                               trace=trace)
    total = sum(float(np.asarray(res.results[b]["out"])[0, 0])
                for b in range(B))
    loss = np.float32(total / float(B * T * H * W))
    return loss, res


def kernel(**inputs):
    loss, _ = run(inputs, trace=False)
    return loss


# revision 3
# speedup vs baseline: 1.0181x; 1.0181x over previous
"""Trainium2 Bass kernel for nn_CAMLocalHead (CAM target + conv head + BCE).

Self-contained: takes FULL inputs, shards batch B=8 across 8 NeuronCores
(one sample per core), runs a Bass/Tile kernel per core, sums the per-core
partial BCE sums on host.
"""
import sys

for _p in ("/opt/trn_rl_repo", "/opt/pypackages"):
    if _p not in sys.path:
        sys.path.append(_p)

import numpy as np
import ml_dtypes

# Problem dims (hardcoded per spec)
B, C, T, H, W = 8, 2048, 16, 7, 7
K, D = 400, 512
N_TOKEN = 392
P = 128
CT = C // P          # 16 c-tiles
DT = D // P          # 4 d-tiles
NH = 2               # spatial halves (t 0..7, 8..15)
TH = T // NH         # 8
PLANE = 81           # 9x9 padded plane
NF = TH * H * W      # 392 positions per half
NPOS = T * H * W     # 784
PADN = 7 * P         # 896 (784 padded to 7 chunks of 128)
NEG = -1.0e30

_cache = {}


def _build_nc():
    import concourse.bacc as bacc
    import concourse.mybir as mybir
    from concourse import tile

    f32 = mybir.dt.float32
    bf16 = mybir.dt.bfloat16
    AX = mybir.AxisListType.X
    OP = mybir.AluOpType
    AF = mybir.ActivationFunctionType

    nc = bacc.Bacc(trn_type="TRN2")

    xpad_d = nc.dram_tensor("xpad", [CT, P, T * PLANE], bf16, kind="ExternalInput")
    wt_d = nc.dram_tensor("wt", [DT, 9, P, CT * P], bf16, kind="ExternalInput")
    proj_d = nc.dram_tensor("proj", [K, C], bf16, kind="ExternalInput")
    xfp_d = nc.dram_tensor("xfp", [1, K], f32, kind="ExternalInput")
    cb_d = nc.dram_tensor("cb", [P, DT], f32, kind="ExternalInput")
    sw_d = nc.dram_tensor("sw", [P, DT], bf16, kind="ExternalInput")
    sb_d = nc.dram_tensor("sb", [1, 1], f32, kind="ExternalInput")
    out_d = nc.dram_tensor("out", [1, 1], f32, kind="ExternalOutput")

    with tile.TileContext(nc) as tc:
        with (
            tc.tile_pool(name="const", bufs=1) as cp,
            tc.tile_pool(name="wp", bufs=3) as wp,
            tc.tile_pool(name="rp", bufs=4) as rp,
            tc.tile_pool(name="cps", bufs=2, space="PSUM") as cps,
            tc.tile_pool(name="sps", bufs=1, space="PSUM") as sps,
            tc.tile_pool(name="mps", bufs=2, space="PSUM") as mps,
        ):
            # ---------- constants / inputs resident in SBUF ----------
            xall = cp.tile([P, CT * T * PLANE], bf16)
            for ct in range(CT):
                nc.sync.dma_start(
                    xall[:, ct * T * PLANE:(ct + 1) * T * PLANE], xpad_d[ct])

            cb_sb = cp.tile([P, DT], f32)
            nc.sync.dma_start(cb_sb[:], cb_d[:])
            sw_sb = cp.tile([P, DT], bf16)
            nc.sync.dma_start(sw_sb[:], sw_d[:])
            sb_sb = cp.tile([1, 1], f32)
            nc.sync.dma_start(sb_sb[:], sb_d[:])

            ones11 = cp.tile([1, 1], f32)
            nc.vector.memset(ones11[:], 1.0)
            ones_row = cp.tile([1, P], f32)
            nc.vector.memset(ones_row[:], 1.0)
            ones_col = cp.tile([P, 1], f32)
            nc.vector.memset(ones_col[:], 1.0)

            xv = xall[:].rearrange(
                "p (ct t h w) -> p ct t h w", ct=CT, t=T, h=9, w=9)

            def xview(ct, tap, nh):
                dh, dw = tap // 3, tap % 3
                return xv[:, ct, nh * TH:(nh + 1) * TH, dh:dh + 7, dw:dw + 7]

            # ---------- CAM front-end (emitted between conv dt0 and dt1
            # so the DMA preamble overlaps PE work) ----------
            fe = {}

            def emit_frontend():
                proj_sb = cp.tile([P, 4 * C], bf16)
                for kc in range(4):
                    kcnt = min(P, K - kc * P)
                    nc.sync.dma_start(
                        proj_sb[0:kcnt, kc * C:(kc + 1) * C],
                        proj_d[kc * P:kc * P + kcnt, :])
                xfp = cp.tile([1, K], f32)
                nc.sync.dma_start(xfp[:], xfp_d[:])

                # argmax class via one-hot (sigmoid monotonic -> argmax on raw)
                m = cp.tile([1, 1], f32)
                nc.vector.reduce_max(m[:], xfp[:], axis=AX)
                oh = cp.tile([1, 4 * P], f32)
                nc.vector.memset(oh[:], 0.0)
                nc.vector.tensor_scalar(oh[0:1, 0:K], xfp[:], m[:], None,
                                        op0=OP.is_equal)
                ohT_ps = mps.tile([P, 4], f32, tag="mp")
                for i in range(4):
                    nc.tensor.transpose(ohT_ps[:, i:i + 1],
                                        oh[0:1, i * P:(i + 1) * P], ones11[:])
                ohT = cp.tile([P, 4], bf16)
                nc.vector.tensor_copy(ohT[:], ohT_ps[:])

                # w_selT[c] = proj_weight[top_cls, c], [128, CT] (c-tile cols)
                wps = mps.tile([P, CT], f32, tag="mp")
                for ct in range(CT):
                    for kc in range(4):
                        kcnt = min(P, K - kc * P)
                        nc.tensor.matmul(
                            wps[:, ct:ct + 1],
                            proj_sb[0:kcnt,
                                    kc * C + ct * P:kc * C + (ct + 1) * P],
                            ohT[0:kcnt, kc:kc + 1],
                            start=(kc == 0), stop=(kc == 3))
                wsel = cp.tile([P, CT], bf16)
                nc.vector.tensor_copy(wsel[:], wps[:])

                # cam[1, 784] = w_sel @ x  (center view of padded x)
                cam_ps = [mps.tile([1, NF], f32, tag="mp", name=f"cam_ps{_nh}")
                          for _nh in range(NH)]
                for nh in range(NH):
                    for ct in range(CT):
                        nc.tensor.matmul(
                            cam_ps[nh][:], wsel[:, ct:ct + 1],
                            xview(ct, 4, nh),
                            start=(ct == 0), stop=(ct == CT - 1))
                cam_row = cp.tile([1, PADN], f32)
                for nh in range(NH):
                    nc.vector.tensor_copy(
                        cam_row[0:1, nh * NF:(nh + 1) * NF], cam_ps[nh][:])

                cmin = cp.tile([1, 1], f32)
                cmax = cp.tile([1, 1], f32)
                nc.vector.tensor_reduce(cmin[:], cam_row[0:1, 0:NPOS],
                                        axis=AX, op=OP.min)
                nc.vector.reduce_max(cmax[:], cam_row[0:1, 0:NPOS], axis=AX)
                rng_t = cp.tile([1, 1], f32)
                nc.vector.tensor_scalar(rng_t[:], cmax[:], cmin[:], None,
                                        op0=OP.subtract)
                inv = cp.tile([1, 1], f32)
                nc.vector.reciprocal(inv[:], rng_t[:])

                camn = cp.tile([1, PADN], f32)
                nc.vector.memset(camn[:], NEG)
                nc.vector.tensor_scalar(camn[0:1, 0:NPOS],
                                        cam_row[0:1, 0:NPOS],
                                        cmin[:], inv[:],
                                        op0=OP.subtract, op1=OP.mult)

                # broadcast camn across partitions: camB[128, 784]
                camB = cp.tile([P, NPOS], f32)
                for nh in range(NH):
                    cb_ps = mps.tile([P, NF], f32, tag="mp")
                    nc.tensor.matmul(cb_ps[:], ones_row[:],
                                     camn[0:1, nh * NF:(nh + 1) * NF],
                                     start=True, stop=True)
                    nc.vector.tensor_copy(
                        camB[:, nh * NF:(nh + 1) * NF], cb_ps[:])

                # camn in partition layout [128, 7]
                cnp_ps = mps.tile([P, 7], f32, tag="mp")
                for a in range(7):
                    nc.tensor.transpose(cnp_ps[:, a:a + 1],
                                        camn[0:1, a * P:(a + 1) * P],
                                        ones11[:])
                camnP = cp.tile([P, 7], f32)
                nc.vector.tensor_copy(camnP[:], cnp_ps[:])

                # rank[p,a] = #{j: camn[j] >= camn[p,a]}; top-392 = rank<=392
                ge = cp.tile([P, NPOS], f32)
                rank = cp.tile([P, 7], f32)
                for a in range(7):
                    nc.vector.tensor_scalar(ge[:], camB[:],
                                            camnP[:, a:a + 1],
                                            None, op0=OP.is_ge, op1=OP.add,
                                            accum_out=rank[:, a:a + 1])
                maskP = cp.tile([P, 7], f32)
                nc.vector.tensor_scalar(maskP[:], rank[:], float(N_TOKEN),
                                        None, op0=OP.is_le)
                yP = cp.tile([P, 7], f32)
                nc.vector.tensor_mul(yP[:], maskP[:], camnP[:])
                fe["yP"] = yP

            # ---------- conv main loop ----------
            s_ps = [sps.tile([1, NF], f32, tag=f"s{nh}", name=f"s_ps{nh}")
                    for nh in range(NH)]

            def emit_conv_dt(dt):
                ps = [cps.tile([P, NF], f32, tag=f"cv{nh}",
                               name=f"ps{dt}_{nh}")
                      for nh in range(NH)]
                for tap in range(9):
                    w_all = wp.tile([P, CT * P], bf16, name="w_all")
                    nc.sync.dma_start(w_all[:], wt_d[dt, tap])
                    for ct in range(CT):
                        lhsT = w_all[:, ct * P:(ct + 1) * P]
                        for nh in range(NH):
                            nc.tensor.matmul(
                                ps[nh][:], lhsT, xview(ct, tap, nh),
                                start=(tap == 0 and ct == 0),
                                stop=(tap == 8 and ct == CT - 1))
                for nh in range(NH):
                    relu_t = rp.tile([P, NF], bf16, name="relu_t")
                    nc.scalar.activation(relu_t[:], ps[nh][:], AF.Relu,
                                         bias=cb_sb[:, dt:dt + 1])
                    nc.tensor.matmul(s_ps[nh][:], sw_sb[:, dt:dt + 1],
                                     relu_t[:],
                                     start=(dt == 0), stop=(dt == DT - 1))

            emit_conv_dt(0)
            emit_frontend()
            yP = fe["yP"]
            for _dt in range(1, DT):
                emit_conv_dt(_dt)

            # ---------- epilogue: BCE = sum softplus(xcam) - sum xcam*y ----
            xcam_row = cp.tile([1, PADN], f32)
            nc.vector.memset(xcam_row[:], 0.0)
            for nh in range(NH):
                nc.vector.tensor_scalar(
                    xcam_row[0:1, nh * NF:(nh + 1) * NF], s_ps[nh][:],
                    sb_sb[:], None, op0=OP.add)

            et = cp.tile([1, NPOS], f32)
            nc.scalar.activation(et[:], xcam_row[0:1, 0:NPOS], AF.Exp)
            sp = cp.tile([1, NPOS], f32)
            sp_sum = cp.tile([1, 1], f32)
            nc.scalar.activation(sp[:], et[:], AF.Ln, bias=1.0,
                                 accum_out=sp_sum[:])

            xcp_ps = mps.tile([P, 7], f32, tag="mp")
            for a in range(7):
                nc.tensor.transpose(xcp_ps[:, a:a + 1],
                                    xcam_row[0:1, a * P:(a + 1) * P],
                                    ones11[:])
            xcamP = cp.tile([P, 7], f32)
            nc.vector.tensor_copy(xcamP[:], xcp_ps[:])

            prodP = cp.tile([P, 7], f32)
            nc.vector.tensor_mul(prodP[:], yP[:], xcamP[:])
            partial = cp.tile([P, 1], f32)
            nc.vector.reduce_sum(partial[:], prodP[:], axis=AX)

            dot_ps = mps.tile([1, 1], f32, tag="mp")
            nc.tensor.matmul(dot_ps[:], ones_col[:], partial[:],
                             start=True, stop=True)

            final = cp.tile([1, 1], f32)
            nc.vector.tensor_scalar(final[:], dot_ps[:], -1.0, sp_sum[:],
                                    op0=OP.mult, op1=OP.add)
            nc.sync.dma_start(out_d[:], final[:])

    nc.compile()
    return nc


def _prep_in_maps(x, x_fpv_pred, proj_weight, conv1_w, conv1_b, score_w,
                  score_b):
    bf16 = ml_dtypes.bfloat16
    xr = np.asarray(x, np.float32).reshape(B, CT, P, T, H, W)
    xpad = np.zeros((B, CT, P, T, 9, 9), dtype=bf16)
    xpad[:, :, :, :, 1:8, 1:8] = xr.astype(bf16)
    xpad = np.ascontiguousarray(xpad.reshape(B, CT, P, T * PLANE))

    w9 = np.asarray(conv1_w, np.float32).reshape(D, C, 9)
    # wt[dt, tap, p, ct*P + q] = conv1_w[dt*P+q, ct*P+p, tap]
    wt = np.ascontiguousarray(
        w9.reshape(DT, P, CT, P, 9).transpose(0, 4, 3, 2, 1)
        .reshape(DT, 9, P, CT * P)).astype(bf16)

    proj_bf = np.asarray(proj_weight, np.float32).astype(bf16)
    cb = np.ascontiguousarray(
        np.asarray(conv1_b, np.float32).reshape(DT, P).T)
    sw = np.ascontiguousarray(
        np.asarray(score_w, np.float32).reshape(DT, P).T).astype(bf16)
    sb = np.asarray(score_b, np.float32).reshape(1, 1)
    xfp = np.asarray(x_fpv_pred, np.float32)

    in_maps = []
    for b in range(B):
        in_maps.append({
            "xpad": xpad[b],
            "wt": wt,
            "proj": proj_bf,
            "xfp": np.ascontiguousarray(xfp[b:b + 1]),
            "cb": cb,
            "sw": sw,
            "sb": sb,
        })
    return in_maps


def run(inputs, trace=False):
    """Build (cached), run on 8 cores, return (loss, BassKernelResults)."""
    from concourse.bass_utils import run_bass_kernel_spmd

    if "nc" not in _cache:
        _cache["nc"] = _build_nc()
    nc = _cache["nc"]
    in_maps = _prep_in_maps(**inputs)
    res = run_bass_kernel_spmd(nc, in_maps, core_ids=list(range(B)),
                               trace=trace)
    total = sum(float(np.asarray(res.results[b]["out"])[0, 0])
                for b in range(B))
    loss = np.float32(total / float(B * T * H * W))
    return loss, res


def kernel(**inputs):
    loss, _ = run(inputs, trace=False)
    return loss
